# revision 9
# baseline (speedup 1.0000x reference)
"""Trainium2 Bass kernel for nn_HINGE_56985626083396 (dense_cnn) — v2.

Data-parallel over batch on 8 NeuronCores. Differences vs v1:
  * per-core vocab compaction on host: each core references at most
    5*bl role rows / 6*bl value rows (< 32768), so indices fit int16
    against a per-core compacted table -> ONE dma_gather per table per
    chunk (v1 needed lo/hi double gathers + DVE merge adds).
    (num_swdge_queues=2 silently corrupts gathers on HW -- single queue.)
  * batch sums for BN means come free from the PSUM->SBUF combines
    (ACT copy accum / DVE scalar_tensor_tensor accum), replacing v1's
    activation-engine copy-accum over inputs + S1 matmul machinery.
  * nb=1024 chunks; window outputs packed per (chunk,fc) into one
    [128, 6*nb] tile -> ONE spill DMA per (chunk,fc), one reload per
    (block,fc) in phase 2.

Per core:
  phase 1 (per 1024-row chunk): gather embeddings (bf16, transposed
    layout); per fc quarter: conv1 windows o0/o1 formed whole in PSUM
    (shared W1r*fr0 term recomputed), conv2 shared B2 + per-k partial
    sums on PE; ACT/DVE combine to bf16 + per-(slot,fc) batch sums;
    squares with accum (split ACT/DVE) -> sum-of-squares; one spill DMA.
  AllReduce (8 cores) of the [128,48] BN sufficient statistics.
  phase 2 (per 1024-row block): reload, BN affine on ACT, min over 6
    windows + relu on DVE, FC dot on PE -> [bl] f32.
"""

import numpy as np
import ml_dtypes

from concourse import bass, bacc, mybir
import concourse.tile as tile
from concourse.bass_utils import run_bass_kernel_spmd
from concourse.library_config import mlp

CORES = 8
B_FULL = 32768
E = 256
F = 512
ARITY = 6
BN_EPS = 1e-5
RSLOTS = 5   # fr0, kr0..kr3     (roles table)
VSLOTS = 6   # fv0, fv1, kv0..3  (values table)

bf16 = mybir.dt.bfloat16
f32 = mybir.dt.float32
i16 = mybir.dt.int16
AF = mybir.ActivationFunctionType
OP = mybir.AluOpType

# weight ids: 0=W1v 1=W1r 2=W2_fv0 3=W2_fr0 4=W2_fv1 5=W2_kr 6=W2_kv
# role slot ids: fr0=0 kr_k=1+k ; value slot ids: fv0=0 fv1=1 kv_k=2+k


def build_nc(bl, nb=1024, use_cc=True, phase2=True):
    """Build the per-core Bass module. bl = per-core batch."""
    nchunk = bl // nb
    nrc = RSLOTS * bl   # compacted roles table rows (upper bound)
    nvc = VSLOTS * bl   # compacted values table rows
    nc = bacc.Bacc("TRN2", target_bir_lowering=False, debug=False,
                   num_devices=CORES)

    r_tbl = nc.dram_tensor("r_tbl", [nrc, E], bf16, kind="ExternalInput")
    v_tbl = nc.dram_tensor("v_tbl", [nvc, E], bf16, kind="ExternalInput")
    # host pre-chunked: [p, (w, kc, fc, j)]
    w_all_d = nc.dram_tensor("w_all", [128, 7 * 2 * 4 * 128], bf16,
                             kind="ExternalInput")
    fcw_d = nc.dram_tensor("fcw", [128, 4], bf16, kind="ExternalInput")
    fcb_d = nc.dram_tensor("fcb", [1, 1], f32, kind="ExternalInput")
    gam_d = nc.dram_tensor("gamma_x", [128, 24], f32, kind="ExternalInput")
    bet_d = nc.dram_tensor("beta_x", [128, 24], f32, kind="ExternalInput")
    NRI = RSLOTS * nb
    NVI = VSLOTS * nb
    WR, WV = NRI // 16, NVI // 16
    idx_d = nc.dram_tensor("idx_all", [128, nchunk * (WR + WV)], i16,
                           kind="ExternalInput")
    out_d = nc.dram_tensor("out", [1, bl], f32, kind="ExternalOutput")
    # [fc][p][(chunk, slot6, nb)] -- one contiguous 12KB/partition write
    # per (chunk, fc)
    spill = nc.dram_tensor("spill", [4, 128, nchunk * 6 * nb], bf16)

    nc.gpsimd.load_library(mlp)

    with tile.TileContext(nc) as tc:
        with tc.tile_pool(name="wp", bufs=1) as wp, \
             tc.tile_pool(name="statp", bufs=1) as statp:
            w_sb = wp.tile([128, 7 * 2 * 4 * 128], bf16)
            nc.sync.dma_start(out=w_sb[:], in_=w_all_d[:])
            fcw_sb = wp.tile([128, 4], bf16)
            nc.sync.dma_start(out=fcw_sb[:], in_=fcw_d[:])
            fcb_sb = wp.tile([1, 1], f32)
            nc.sync.dma_start(out=fcb_sb[:], in_=fcb_d[:])
            gam_sb = wp.tile([128, 24], f32)
            nc.sync.dma_start(out=gam_sb[:], in_=gam_d[:])
            bet_sb = wp.tile([128, 24], f32)
            nc.sync.dma_start(out=bet_sb[:], in_=bet_d[:])

            def w(wi, kc, fc):
                off = ((wi * 2 + kc) * 4 + fc) * 128
                return w_sb[:, off:off + 128]

            # per-(slot6,fc,chunk,half) batch sums; per-(slot6,fc,chunk)
            # sums of squares
            sumc = statp.tile([128, 6 * 4 * nchunk * 2], f32)
            s2c = statp.tile([128, 6 * 4 * nchunk], f32)

            # ---------------- phase 1 ----------------
            with tc.tile_pool(name="idxp", bufs=1) as idxp, \
                 tc.tile_pool(name="xp", bufs=2) as xp, \
                 tc.tile_pool(name="scrp", bufs=2) as scrp, \
                 tc.tile_pool(name="outp", bufs=2) as outp, \
                 tc.tile_pool(name="po", bufs=2, space="PSUM") as po, \
                 tc.tile_pool(name="psg", bufs=2, space="PSUM") as psg:
                idx_sb = idxp.tile([128, nchunk * (WR + WV)], i16)
                nc.sync.dma_start(out=idx_sb[:], in_=idx_d[:])

                for ch in range(nchunk):
                    co = ch * (WR + WV)
                    rl = idx_sb[:, co:co + WR]
                    vl = idx_sb[:, co + WR:co + WR + WV]

                    xr = xp.tile([128, 2, NRI], bf16, tag="xr")
                    nc.gpsimd.dma_gather(xr[:], r_tbl[:], rl, NRI, NRI, E,
                                         transpose=True, single_packet=False)
                    xv = xp.tile([128, 2, NVI], bf16, tag="xv")
                    nc.gpsimd.dma_gather(xv[:], v_tbl[:], vl, NVI, NVI, E,
                                         transpose=True, single_packet=False)

                    def xs_r(s, kc):
                        return xr[:, kc, s * nb:(s + 1) * nb]

                    def xs_v(s, kc):
                        return xv[:, kc, s * nb:(s + 1) * nb]

                    HB = nb // 2  # PSUM ops stay within one 512-f32 bank

                    def mm_group2(t0, t1, units):
                        """units: list of (wi, x-slice-fn, slot). Accumulate
                        sum_u W[u]*x[u] into t0/t1 (batch halves). Both
                        halves run back-to-back under one weight load."""
                        n = len(units) * 2
                        i = 0
                        for (wi, xs, s) in units:
                            for kc in range(2):
                                for h, tt in ((0, t0), (1, t1)):
                                    nc.tensor.matmul(
                                        out=tt[:],
                                        lhsT=w(wi, kc, fc),
                                        rhs=xs(s, kc)[:, h * HB:(h + 1) * HB],
                                        start=(i == 0), stop=(i == n - 1))
                                i += 1

                    for fc in range(4):
                        obuf = outp.tile([128, 6, nb], bf16, tag="obuf")

                        def ob(s6, h):
                            return obuf[:, s6, h * HB:(h + 1) * HB]

                        def scol(s6, h):
                            c = (((s6 * 4 + fc) * nchunk) + ch) * 2 + h
                            return sumc[:, c:c + 1]

                        # conv1 windows formed whole in PSUM:
                        # o0 = W1v*fv0 + W1r*fr0 ; o1 = W1v*fv1 + W1r*fr0
                        # (shared W1r*fr0 recomputed -- cheaper than a third
                        #  PSUM tile + DVE combine under the bank budget)
                        for s6 in range(2):
                            t0 = po.tile([128, HB], f32, tag="po0")
                            t1 = po.tile([128, HB], f32, tag="po1")
                            mm_group2(t0, t1, [(0, xs_v, s6), (1, xs_r, 0)])
                            for h, tt in ((0, t0), (1, t1)):
                                nc.scalar.activation(
                                    out=ob(s6, h), in_=tt[:], func=AF.Copy,
                                    accum_out=scol(s6, h))

                        # conv2 shared part: B2 = W2fv0*fv0 + W2fr0*fr0 + W2fv1*fv1
                        b2sb = scrp.tile([128, nb], f32, tag="b2sb")
                        t0 = po.tile([128, HB], f32, tag="po0")
                        t1 = po.tile([128, HB], f32, tag="po1")
                        mm_group2(t0, t1, [(2, xs_v, 0), (3, xs_r, 0),
                                           (4, xs_v, 1)])
                        for h, tt in ((0, t0), (1, t1)):
                            nc.scalar.copy(out=b2sb[:, h * HB:(h + 1) * HB],
                                           in_=tt[:])

                        # conv2 per-k: g_k = W2kr*kr_k + W2kv*kv_k (+B2)
                        for k in range(4):
                            g0 = psg.tile([128, HB], f32, tag="g0")
                            g1 = psg.tile([128, HB], f32, tag="g1")
                            mm_group2(g0, g1, [(5, xs_r, 1 + k),
                                               (6, xs_v, 2 + k)])
                            for h, gg in ((0, g0), (1, g1)):
                                nc.vector.scalar_tensor_tensor(
                                    out=ob(2 + k, h), in0=gg[:], scalar=1.0,
                                    in1=b2sb[:, h * HB:(h + 1) * HB],
                                    op0=OP.mult, op1=OP.add,
                                    accum_out=scol(2 + k, h))

                        # sums of squares (ACT; tensor_tensor_reduce and
                        # Identity-with-AP-affine both wedge real HW)
                        for s6 in range(6):
                            col = (s6 * 4 + fc) * nchunk + ch
                            sqs = scrp.tile([128, nb], bf16, tag="sqs")
                            nc.scalar.activation(
                                out=sqs[:], in_=obuf[:, s6, :], func=AF.Square,
                                accum_out=s2c[:, col:col + 1])

                        eng = nc.sync if fc % 2 == 0 else nc.scalar
                        eng.dma_start(
                            out=spill[fc, :, (ch * 6) * nb:(ch * 6 + 6) * nb],
                            in_=obuf[:])

            # ---------------- stats + allreduce ----------------
            with tc.tile_pool(name="fsp", bufs=1) as fsp, \
                 tc.tile_pool(name="dramp", bufs=1, space="DRAM") as dramp:
                stats = fsp.tile([128, 48], f32)
                nc.vector.tensor_reduce(
                    out=stats[:, 0:24],
                    in_=sumc[:].rearrange("p (a c) -> p a c", c=nchunk * 2),
                    axis=mybir.AxisListType.X, op=OP.add)
                nc.vector.tensor_reduce(
                    out=stats[:, 24:48],
                    in_=s2c[:].rearrange("p (a c) -> p a c", c=nchunk),
                    axis=mybir.AxisListType.X, op=OP.add)

                cc_in = dramp.tile([128, 48], f32)
                cc_out = dramp.tile([128, 48], f32)
                nc.sync.dma_start(out=cc_in[:], in_=stats[:])
                ar = fsp.tile([128, 48], f32)
                if use_cc:
                    nc.gpsimd.collective_compute(
                        "AllReduce", OP.add,
                        replica_groups=[list(range(CORES))],
                        ins=[cc_in[:].opt()], outs=[cc_out[:].opt()])
                    nc.sync.dma_start(out=ar[:], in_=cc_out[:])
                else:
                    nc.sync.dma_start(out=ar[:], in_=cc_in[:])

                inv_n = 1.0 / (bl * CORES) if use_cc else 1.0 / bl
                mean = fsp.tile([128, 24], f32)
                nc.scalar.mul(out=mean[:], in_=ar[:, 0:24], mul=inv_n)
                ex2 = fsp.tile([128, 24], f32)
                nc.scalar.mul(out=ex2[:], in_=ar[:, 24:48], mul=inv_n)
                var = fsp.tile([128, 24], f32)
                nc.vector.tensor_tensor(out=var[:], in0=mean[:], in1=mean[:],
                                        op=OP.mult)
                nc.vector.tensor_tensor(out=var[:], in0=ex2[:], in1=var[:],
                                        op=OP.subtract)
                nc.vector.tensor_scalar_add(out=var[:], in0=var[:],
                                            scalar1=BN_EPS)
                std = fsp.tile([128, 24], f32)
                nc.scalar.activation(out=std[:], in_=var[:], func=AF.Sqrt,
                                     bias=0.0, scale=1.0)
                rstd = fsp.tile([128, 24], f32)
                nc.vector.reciprocal(out=rstd[:], in_=std[:])
                avec = fsp.tile([128, 24], f32)
                nc.vector.tensor_tensor(out=avec[:], in0=rstd[:], in1=gam_sb[:],
                                        op=OP.mult)
                cvec = fsp.tile([128, 24], f32)
                nc.vector.tensor_tensor(out=cvec[:], in0=mean[:], in1=avec[:],
                                        op=OP.mult)
                nc.vector.tensor_tensor(out=cvec[:], in0=bet_sb[:], in1=cvec[:],
                                        op=OP.subtract)

                # ---------------- phase 2 ----------------
                with tc.tile_pool(name="ldp", bufs=2) as ldp, \
                     tc.tile_pool(name="yp", bufs=4) as yp, \
                     tc.tile_pool(name="mp", bufs=2) as mp, \
                     tc.tile_pool(name="mrp", bufs=8) as mrp, \
                     tc.tile_pool(name="orow", bufs=1) as orow, \
                     tc.tile_pool(name="ps2", bufs=2, space="PSUM") as ps2:
                    outrow = orow.tile([1, bl], f32)
                    if not phase2:
                        dbg = fsp.tile([1, bl], f32)
                        nc.vector.tensor_copy(out=dbg[:, 0:24], in_=avec[:1, :])
                        nc.sync.dma_start(out=out_d[:], in_=dbg[:])
                    for ch in (range(nchunk) if phase2 else []):
                        bo = ch * nb
                        minres = []
                        for fc in range(4):
                            ld = ldp.tile([128, 6, nb], bf16, tag="ld")
                            eng = nc.sync if fc % 2 == 0 else nc.scalar
                            eng.dma_start(
                                out=ld[:],
                                in_=spill[fc, :, (ch * 6) * nb:(ch * 6 + 6) * nb])
                            acc = None
                            for s6 in range(6):
                                col = s6 * 4 + fc
                                y = yp.tile([128, nb], bf16, tag="y")
                                # BN affine split across DVE / GPSIMD
                                aeng = nc.vector if s6 % 2 == 0 else nc.gpsimd
                                aeng.tensor_scalar(
                                    out=y[:], in0=ld[:, s6, :],
                                    scalar1=avec[:, col:col + 1],
                                    scalar2=cvec[:, col:col + 1],
                                    op0=OP.mult, op1=OP.add)
                                if acc is None:
                                    acc = y
                                elif s6 < 5:
                                    m2 = mp.tile([128, nb], bf16, tag="m")
                                    nc.vector.tensor_tensor(
                                        out=m2[:], in0=acc[:], in1=y[:],
                                        op=OP.min)
                                    acc = m2
                                else:
                                    mr = mrp.tile([128, nb], bf16, tag="mr")
                                    nc.vector.tensor_tensor(
                                        out=mr[:], in0=acc[:], in1=y[:],
                                        op=OP.min)
                                    minres.append(mr)
                        fcsub = 512
                        for sub in range(0, nb, fcsub):
                            pfc = ps2.tile([1, fcsub], f32, tag="pfc")
                            for fc in range(4):
                                relu = mrp.tile([128, fcsub], bf16, tag="relu")
                                nc.scalar.activation(
                                    out=relu[:],
                                    in_=minres[fc][:, sub:sub + fcsub],
                                    func=AF.Relu, bias=0.0, scale=1.0)
                                nc.tensor.matmul(out=pfc[:],
                                                 lhsT=fcw_sb[:, fc:fc + 1],
                                                 rhs=relu[:],
                                                 start=(fc == 0), stop=(fc == 3))
                            nc.vector.tensor_scalar_add(
                                out=outrow[:, bo + sub:bo + sub + fcsub],
                                in0=pfc[:], scalar1=fcb_sb[:1, :1])
                    if phase2:
                        nc.sync.dma_start(out=out_d[:], in_=outrow[:])

    nc.compile()
    return nc


# ---------------- host side ----------------

def _wrap16(a):
    """int16 index array -> [128, n/16] wrapped layout."""
    a = a.astype(np.int16)
    w = a.reshape(-1, 16).T
    return np.tile(w, (8, 1))


def _prep_idx_packed(r_idx, v_idx, nb, nchunk):
    """r_idx/v_idx: [bl, nslots] int (< 32768). -> [128, nchunk*(WR+WV)]
    where per chunk the roles block then values block are laid out
    column-contiguous (wrapped in 16 partitions, replicated to 128)."""
    cols = []
    for ch in range(nchunk):
        for slots_idx in (r_idx, v_idx):
            blk = slots_idx[ch * nb:(ch + 1) * nb]   # [nb, nslots]
            flat = blk.T.reshape(-1)                  # slot-major
            cols.append(_wrap16(flat))
    return np.concatenate(cols, 1)


def _chunk_weights(conv1_w, conv2_w):
    """-> [128, 7*2*4*128] bf16 host-chunked transposed weights."""
    ws = [conv1_w[:, 0, :], conv1_w[:, 1, :],
          conv2_w[:, 0, :], conv2_w[:, 1, :], conv2_w[:, 2, :],
          conv2_w[:, 3, :], conv2_w[:, 4, :]]
    out = np.empty((128, 7, 2, 4, 128), dtype=ml_dtypes.bfloat16)
    for wi, wm in enumerate(ws):
        wt = np.asarray(wm, np.float32).T           # [E, F]
        c = wt.reshape(2, 128, 4, 128)              # [kc, p, fc, j]
        out[:, wi] = c.transpose(1, 0, 2, 3)
    return out.reshape(128, -1)


def _expand_bn(v1, v2):
    """bn1/bn2 [F] -> [128, 24] per (slot, fc) expanded."""
    out = np.empty((128, 6, 4), np.float32)
    for slot in range(6):
        src = v1 if slot < 2 else v2
        out[:, slot, :] = np.asarray(src, np.float32).reshape(4, 128).T
    return out.reshape(128, 24)


_CACHE = {}


def _get_nc(bl):
    if bl not in _CACHE:
        _CACHE[bl] = build_nc(bl)
    return _CACHE[bl]


def _compact(tbl_bf16, slots_idx, nrows):
    """Compacted per-core table + remapped int16 indices.

    slots_idx: [bl, nslots] int into the full table. Returns
    (table [nrows, E] bf16, idx [bl, nslots] int16).
    """
    uniq, inv = np.unique(slots_idx, return_inverse=True)
    assert uniq.size <= nrows
    tbl_c = np.zeros((nrows, E), dtype=tbl_bf16.dtype)
    tbl_c[:uniq.size] = tbl_bf16[uniq]
    return tbl_c, inv.reshape(slots_idx.shape).astype(np.int16)


def make_in_maps(x_batch, emb_roles, emb_values, conv1_w, conv2_w,
                 bn1_gamma, bn1_beta, bn2_gamma, bn2_beta, fc_w, fc_b,
                 bl, nb=1024):
    nchunk = bl // nb
    er = np.asarray(emb_roles, np.float32).astype(ml_dtypes.bfloat16)
    ev = np.asarray(emb_values, np.float32).astype(ml_dtypes.bfloat16)
    shared = {
        "w_all": _chunk_weights(conv1_w, conv2_w),
        "fcw": np.asarray(fc_w, np.float32).reshape(4, 128).T
                 .astype(ml_dtypes.bfloat16),
        "fcb": np.asarray(fc_b, np.float32).reshape(1, 1),
        "gamma_x": _expand_bn(bn1_gamma, bn2_gamma),
        "beta_x": _expand_bn(bn1_beta, bn2_beta),
    }
    xb = np.asarray(x_batch).astype(np.int64)
    in_maps = []
    for c in range(CORES):
        xs = xb[c * bl:(c + 1) * bl]
        roles = xs[:, 0::2]
        values = xs[:, 1::2]
        r_slots = roles[:, [0, 2, 3, 4, 5]]
        v_slots = values[:, [0, 1, 2, 3, 4, 5]]
        r_tbl, r_idx = _compact(er, r_slots, RSLOTS * bl)
        v_tbl, v_idx = _compact(ev, v_slots, VSLOTS * bl)
        m = dict(shared)
        m.update({
            "r_tbl": r_tbl, "v_tbl": v_tbl,
            "idx_all": _prep_idx_packed(r_idx, v_idx, nb, nchunk),
        })
        in_maps.append(m)
    return in_maps


def kernel(x_batch, arity, emb_roles, emb_values,
           conv1_w, conv1_b, bn1_gamma, bn1_beta,
           conv2_w, conv2_b, bn2_gamma, bn2_beta, fc_w, fc_b):
    # conv biases cancel exactly under training-mode batchnorm.
    bl = np.asarray(x_batch).shape[0] // CORES
    nc = _get_nc(bl)
    in_maps = make_in_maps(x_batch, emb_roles, emb_values, conv1_w, conv2_w,
                           bn1_gamma, bn1_beta, bn2_gamma, bn2_beta,
                           fc_w, fc_b, bl)
    res = run_bass_kernel_spmd(nc, in_maps, core_ids=list(range(CORES)))
    out = np.concatenate([res.results[c]["out"].reshape(bl, 1)
                          for c in range(CORES)], 0)
    return out.astype(np.float32)


# revision 11
# speedup vs baseline: 1.0667x; 1.0667x over previous
"""Trainium2 Bass kernel for nn_HINGE_56985626083396 (dense_cnn) — v2.

Data-parallel over batch on 8 NeuronCores. Differences vs v1:
  * per-core vocab compaction on host: each core references at most
    5*bl role rows / 6*bl value rows (< 32768), so indices fit int16
    against a per-core compacted table -> ONE dma_gather per table per
    chunk (v1 needed lo/hi double gathers + DVE merge adds).
    (num_swdge_queues=2 silently corrupts gathers on HW -- single queue.)
  * batch sums for BN means come free from the PSUM->SBUF combines
    (ACT copy accum / DVE scalar_tensor_tensor accum), replacing v1's
    activation-engine copy-accum over inputs + S1 matmul machinery.
  * nb=1024 chunks; window outputs packed per (chunk,fc) into one
    [128, 6*nb] tile -> ONE spill DMA per (chunk,fc), one reload per
    (block,fc) in phase 2.

Per core:
  phase 1 (per 1024-row chunk): gather embeddings (bf16, transposed
    layout); per fc quarter: conv1 windows o0/o1 formed whole in PSUM
    (shared W1r*fr0 term recomputed), conv2 shared B2 + per-k partial
    sums on PE; ACT/DVE combine to bf16 + per-(slot,fc) batch sums;
    squares with accum (split ACT/DVE) -> sum-of-squares; one spill DMA.
  AllReduce (8 cores) of the [128,48] BN sufficient statistics.
  phase 2 (per 1024-row block): reload, BN affine on ACT, min over 6
    windows + relu on DVE, FC dot on PE -> [bl] f32.
"""

import numpy as np
import ml_dtypes

from concourse import bass, bacc, mybir
import concourse.tile as tile
from concourse.bass_utils import run_bass_kernel_spmd
from concourse.library_config import mlp

CORES = 8
B_FULL = 32768
E = 256
F = 512
ARITY = 6
BN_EPS = 1e-5
RSLOTS = 5   # fr0, kr0..kr3     (roles table)
VSLOTS = 6   # fv0, fv1, kv0..3  (values table)

bf16 = mybir.dt.bfloat16
f32 = mybir.dt.float32
i16 = mybir.dt.int16
AF = mybir.ActivationFunctionType
OP = mybir.AluOpType

# weight ids: 0=W1v 1=W1r 2=W2_fv0 3=W2_fr0 4=W2_fv1 5=W2_kr 6=W2_kv
# role slot ids: fr0=0 kr_k=1+k ; value slot ids: fv0=0 fv1=1 kv_k=2+k


def build_nc(bl, nb=1024, use_cc=True, phase2=True):
    """Build the per-core Bass module. bl = per-core batch."""
    nchunk = bl // nb
    nrc = RSLOTS * bl   # compacted roles table rows (upper bound)
    nvc = VSLOTS * bl   # compacted values table rows
    nc = bacc.Bacc("TRN2", target_bir_lowering=False, debug=False,
                   num_devices=CORES)

    r_tbl = nc.dram_tensor("r_tbl", [nrc, E], bf16, kind="ExternalInput")
    v_tbl = nc.dram_tensor("v_tbl", [nvc, E], bf16, kind="ExternalInput")
    # host pre-chunked: [p, (w, kc, fc, j)]
    w_all_d = nc.dram_tensor("w_all", [128, 7 * 2 * 4 * 128], bf16,
                             kind="ExternalInput")
    fcw_d = nc.dram_tensor("fcw", [128, 4], bf16, kind="ExternalInput")
    fcb_d = nc.dram_tensor("fcb", [1, 1], f32, kind="ExternalInput")
    gam_d = nc.dram_tensor("gamma_x", [128, 24], f32, kind="ExternalInput")
    bet_d = nc.dram_tensor("beta_x", [128, 24], f32, kind="ExternalInput")
    NRI = RSLOTS * nb
    NVI = VSLOTS * nb
    WR, WV = NRI // 16, NVI // 16
    idx_d = nc.dram_tensor("idx_all", [128, nchunk * (WR + WV)], i16,
                           kind="ExternalInput")
    out_d = nc.dram_tensor("out", [1, bl], f32, kind="ExternalOutput")
    # [fc][p][(chunk, slot6, nb)] -- one contiguous 12KB/partition write
    # per (chunk, fc)
    spill = nc.dram_tensor("spill", [4, 128, nchunk * 6 * nb], bf16)

    nc.gpsimd.load_library(mlp)

    with tile.TileContext(nc) as tc:
        with tc.tile_pool(name="wp", bufs=1) as wp, \
             tc.tile_pool(name="statp", bufs=1) as statp:
            w_sb = wp.tile([128, 7 * 2 * 4 * 128], bf16)
            nc.sync.dma_start(out=w_sb[:], in_=w_all_d[:])
            fcw_sb = wp.tile([128, 4], bf16)
            nc.sync.dma_start(out=fcw_sb[:], in_=fcw_d[:])
            fcb_sb = wp.tile([1, 1], f32)
            nc.sync.dma_start(out=fcb_sb[:], in_=fcb_d[:])
            gam_sb = wp.tile([128, 24], f32)
            nc.sync.dma_start(out=gam_sb[:], in_=gam_d[:])
            bet_sb = wp.tile([128, 24], f32)
            nc.sync.dma_start(out=bet_sb[:], in_=bet_d[:])

            def w(wi, kc, fc):
                off = ((wi * 2 + kc) * 4 + fc) * 128
                return w_sb[:, off:off + 128]

            # per-(slot6,fc,chunk,half) batch sums; per-(slot6,fc,chunk)
            # sums of squares
            sumc = statp.tile([128, 6 * 4 * nchunk * 2], f32)
            s2c = statp.tile([128, 6 * 4 * nchunk], f32)

            # ---------------- phase 1 ----------------
            with tc.tile_pool(name="idxp", bufs=1) as idxp, \
                 tc.tile_pool(name="xp", bufs=2) as xp, \
                 tc.tile_pool(name="scrp", bufs=2) as scrp, \
                 tc.tile_pool(name="outp", bufs=3) as outp, \
                 tc.tile_pool(name="po", bufs=2, space="PSUM") as po, \
                 tc.tile_pool(name="psg", bufs=2, space="PSUM") as psg:
                idx_sb = idxp.tile([128, nchunk * (WR + WV)], i16)
                nc.sync.dma_start(out=idx_sb[:], in_=idx_d[:])

                for ch in range(nchunk):
                    co = ch * (WR + WV)
                    rl = idx_sb[:, co:co + WR]
                    vl = idx_sb[:, co + WR:co + WR + WV]

                    xr = xp.tile([128, 2, NRI], bf16, tag="xr")
                    nc.gpsimd.dma_gather(xr[:], r_tbl[:], rl, NRI, NRI, E,
                                         transpose=True, single_packet=False)
                    xv = xp.tile([128, 2, NVI], bf16, tag="xv")
                    nc.gpsimd.dma_gather(xv[:], v_tbl[:], vl, NVI, NVI, E,
                                         transpose=True, single_packet=False)

                    def xs_r(s, kc):
                        return xr[:, kc, s * nb:(s + 1) * nb]

                    def xs_v(s, kc):
                        return xv[:, kc, s * nb:(s + 1) * nb]

                    HB = nb // 2  # PSUM ops stay within one 512-f32 bank

                    def mm_group2(t0, t1, units):
                        """units: list of (wi, x-slice-fn, slot). Accumulate
                        sum_u W[u]*x[u] into t0/t1 (batch halves). Both
                        halves run back-to-back under one weight load."""
                        n = len(units) * 2
                        i = 0
                        for (wi, xs, s) in units:
                            for kc in range(2):
                                for h, tt in ((0, t0), (1, t1)):
                                    nc.tensor.matmul(
                                        out=tt[:],
                                        lhsT=w(wi, kc, fc),
                                        rhs=xs(s, kc)[:, h * HB:(h + 1) * HB],
                                        start=(i == 0), stop=(i == n - 1))
                                i += 1

                    def emit_squares(obuf_prev, fc_prev):
                        # sums of squares, lagged one fc so the ACT queue
                        # never delays the PSUM-freeing copies (ACT is a
                        # strict 8-deep FIFO).  (tensor_tensor_reduce and
                        # Identity-with-AP-affine both wedge real HW.)
                        for s6 in range(6):
                            col = (s6 * 4 + fc_prev) * nchunk + ch
                            sqs = scrp.tile([128, nb], bf16, tag="sqs")
                            nc.scalar.activation(
                                out=sqs[:], in_=obuf_prev[:, s6, :],
                                func=AF.Square,
                                accum_out=s2c[:, col:col + 1])

                    prev = None
                    for fc in range(4):
                        obuf = outp.tile([128, 6, nb], bf16, tag="obuf")

                        def ob(s6, h):
                            return obuf[:, s6, h * HB:(h + 1) * HB]

                        def scol(s6, h):
                            c = (((s6 * 4 + fc) * nchunk) + ch) * 2 + h
                            return sumc[:, c:c + 1]

                        # conv1 windows formed whole in PSUM:
                        # o0 = W1v*fv0 + W1r*fr0 ; o1 = W1v*fv1 + W1r*fr0
                        # (shared W1r*fr0 recomputed -- cheaper than a third
                        #  PSUM tile + DVE combine under the bank budget)
                        for s6 in range(2):
                            t0 = po.tile([128, HB], f32, tag="po0")
                            t1 = po.tile([128, HB], f32, tag="po1")
                            mm_group2(t0, t1, [(0, xs_v, s6), (1, xs_r, 0)])
                            for h, tt in ((0, t0), (1, t1)):
                                nc.vector.tensor_scalar(
                                    out=ob(s6, h), in0=tt[:],
                                    scalar1=1.0, scalar2=0.0,
                                    op0=OP.mult, op1=OP.add,
                                    accum_out=scol(s6, h))

                        # conv2 shared part: B2 = W2fv0*fv0 + W2fr0*fr0 + W2fv1*fv1
                        b2sb = scrp.tile([128, nb], f32, tag="b2sb")
                        t0 = po.tile([128, HB], f32, tag="po0")
                        t1 = po.tile([128, HB], f32, tag="po1")
                        mm_group2(t0, t1, [(2, xs_v, 0), (3, xs_r, 0),
                                           (4, xs_v, 1)])
                        for h, tt in ((0, t0), (1, t1)):
                            nc.scalar.copy(out=b2sb[:, h * HB:(h + 1) * HB],
                                           in_=tt[:])
                        if prev is not None:
                            emit_squares(*prev)

                        # conv2 per-k: g_k = W2kr*kr_k + W2kv*kv_k (+B2)
                        for k in range(4):
                            g0 = psg.tile([128, HB], f32, tag="g0")
                            g1 = psg.tile([128, HB], f32, tag="g1")
                            mm_group2(g0, g1, [(5, xs_r, 1 + k),
                                               (6, xs_v, 2 + k)])
                            for h, gg in ((0, g0), (1, g1)):
                                nc.vector.scalar_tensor_tensor(
                                    out=ob(2 + k, h), in0=gg[:], scalar=1.0,
                                    in1=b2sb[:, h * HB:(h + 1) * HB],
                                    op0=OP.mult, op1=OP.add,
                                    accum_out=scol(2 + k, h))

                        eng = nc.sync if fc % 2 == 0 else nc.scalar
                        eng.dma_start(
                            out=spill[fc, :, (ch * 6) * nb:(ch * 6 + 6) * nb],
                            in_=obuf[:])
                        prev = (obuf, fc)
                    emit_squares(*prev)

            # ---------------- stats + allreduce ----------------
            with tc.tile_pool(name="fsp", bufs=1) as fsp, \
                 tc.tile_pool(name="dramp", bufs=1, space="DRAM") as dramp:
                stats = fsp.tile([128, 48], f32)
                nc.vector.tensor_reduce(
                    out=stats[:, 0:24],
                    in_=sumc[:].rearrange("p (a c) -> p a c", c=nchunk * 2),
                    axis=mybir.AxisListType.X, op=OP.add)
                nc.vector.tensor_reduce(
                    out=stats[:, 24:48],
                    in_=s2c[:].rearrange("p (a c) -> p a c", c=nchunk),
                    axis=mybir.AxisListType.X, op=OP.add)

                cc_in = dramp.tile([128, 48], f32)
                cc_out = dramp.tile([128, 48], f32)
                nc.sync.dma_start(out=cc_in[:], in_=stats[:])
                ar = fsp.tile([128, 48], f32)
                if use_cc:
                    nc.gpsimd.collective_compute(
                        "AllReduce", OP.add,
                        replica_groups=[list(range(CORES))],
                        ins=[cc_in[:].opt()], outs=[cc_out[:].opt()])
                    nc.sync.dma_start(out=ar[:], in_=cc_out[:])
                else:
                    nc.sync.dma_start(out=ar[:], in_=cc_in[:])

                inv_n = 1.0 / (bl * CORES) if use_cc else 1.0 / bl
                mean = fsp.tile([128, 24], f32)
                nc.scalar.mul(out=mean[:], in_=ar[:, 0:24], mul=inv_n)
                ex2 = fsp.tile([128, 24], f32)
                nc.scalar.mul(out=ex2[:], in_=ar[:, 24:48], mul=inv_n)
                var = fsp.tile([128, 24], f32)
                nc.vector.tensor_tensor(out=var[:], in0=mean[:], in1=mean[:],
                                        op=OP.mult)
                nc.vector.tensor_tensor(out=var[:], in0=ex2[:], in1=var[:],
                                        op=OP.subtract)
                nc.vector.tensor_scalar_add(out=var[:], in0=var[:],
                                            scalar1=BN_EPS)
                std = fsp.tile([128, 24], f32)
                nc.scalar.activation(out=std[:], in_=var[:], func=AF.Sqrt,
                                     bias=0.0, scale=1.0)
                rstd = fsp.tile([128, 24], f32)
                nc.vector.reciprocal(out=rstd[:], in_=std[:])
                avec = fsp.tile([128, 24], f32)
                nc.vector.tensor_tensor(out=avec[:], in0=rstd[:], in1=gam_sb[:],
                                        op=OP.mult)
                cvec = fsp.tile([128, 24], f32)
                nc.vector.tensor_tensor(out=cvec[:], in0=mean[:], in1=avec[:],
                                        op=OP.mult)
                nc.vector.tensor_tensor(out=cvec[:], in0=bet_sb[:], in1=cvec[:],
                                        op=OP.subtract)

                # ---------------- phase 2 ----------------
                with tc.tile_pool(name="ldp", bufs=3) as ldp, \
                     tc.tile_pool(name="yp", bufs=4) as yp, \
                     tc.tile_pool(name="mp", bufs=2) as mp, \
                     tc.tile_pool(name="mrp", bufs=8) as mrp, \
                     tc.tile_pool(name="orow", bufs=1) as orow, \
                     tc.tile_pool(name="ps2", bufs=2, space="PSUM") as ps2:
                    outrow = orow.tile([1, bl], f32)
                    if not phase2:
                        dbg = fsp.tile([1, bl], f32)
                        nc.vector.tensor_copy(out=dbg[:, 0:24], in_=avec[:1, :])
                        nc.sync.dma_start(out=out_d[:], in_=dbg[:])
                    for ch in (range(nchunk) if phase2 else []):
                        bo = ch * nb
                        minres = []
                        for fc in range(4):
                            ld = ldp.tile([128, 6, nb], bf16, tag="ld")
                            eng = nc.sync if fc % 2 == 0 else nc.scalar
                            eng.dma_start(
                                out=ld[:],
                                in_=spill[fc, :, (ch * 6) * nb:(ch * 6 + 6) * nb])
                            acc = None
                            for s6 in range(6):
                                col = s6 * 4 + fc
                                y = yp.tile([128, nb], bf16, tag="y")
                                # BN affine split across DVE / GPSIMD
                                aeng = nc.vector if s6 % 2 == 0 else nc.gpsimd
                                aeng.tensor_scalar(
                                    out=y[:], in0=ld[:, s6, :],
                                    scalar1=avec[:, col:col + 1],
                                    scalar2=cvec[:, col:col + 1],
                                    op0=OP.mult, op1=OP.add)
                                if acc is None:
                                    acc = y
                                elif s6 < 5:
                                    m2 = mp.tile([128, nb], bf16, tag="m")
                                    nc.vector.tensor_tensor(
                                        out=m2[:], in0=acc[:], in1=y[:],
                                        op=OP.min)
                                    acc = m2
                                else:
                                    mr = mrp.tile([128, nb], bf16, tag="mr")
                                    nc.vector.tensor_tensor(
                                        out=mr[:], in0=acc[:], in1=y[:],
                                        op=OP.min)
                                    minres.append(mr)
                        fcsub = 512
                        for sub in range(0, nb, fcsub):
                            pfc = ps2.tile([1, fcsub], f32, tag="pfc")
                            for fc in range(4):
                                relu = mrp.tile([128, fcsub], bf16, tag="relu")
                                nc.scalar.activation(
                                    out=relu[:],
                                    in_=minres[fc][:, sub:sub + fcsub],
                                    func=AF.Relu, bias=0.0, scale=1.0)
                                nc.tensor.matmul(out=pfc[:],
                                                 lhsT=fcw_sb[:, fc:fc + 1],
                                                 rhs=relu[:],
                                                 start=(fc == 0), stop=(fc == 3))
                            nc.vector.tensor_scalar_add(
                                out=outrow[:, bo + sub:bo + sub + fcsub],
                                in0=pfc[:], scalar1=fcb_sb[:1, :1])
                    if phase2:
                        nc.sync.dma_start(out=out_d[:], in_=outrow[:])

    nc.compile()
    return nc


# ---------------- host side ----------------

def _wrap16(a):
    """int16 index array -> [128, n/16] wrapped layout."""
    a = a.astype(np.int16)
    w = a.reshape(-1, 16).T
    return np.tile(w, (8, 1))


def _prep_idx_packed(r_idx, v_idx, nb, nchunk):
    """r_idx/v_idx: [bl, nslots] int (< 32768). -> [128, nchunk*(WR+WV)]
    where per chunk the roles block then values block are laid out
    column-contiguous (wrapped in 16 partitions, replicated to 128)."""
    cols = []
    for ch in range(nchunk):
        for slots_idx in (r_idx, v_idx):
            blk = slots_idx[ch * nb:(ch + 1) * nb]   # [nb, nslots]
            flat = blk.T.reshape(-1)                  # slot-major
            cols.append(_wrap16(flat))
    return np.concatenate(cols, 1)


def _chunk_weights(conv1_w, conv2_w):
    """-> [128, 7*2*4*128] bf16 host-chunked transposed weights."""
    ws = [conv1_w[:, 0, :], conv1_w[:, 1, :],
          conv2_w[:, 0, :], conv2_w[:, 1, :], conv2_w[:, 2, :],
          conv2_w[:, 3, :], conv2_w[:, 4, :]]
    out = np.empty((128, 7, 2, 4, 128), dtype=ml_dtypes.bfloat16)
    for wi, wm in enumerate(ws):
        wt = np.asarray(wm, np.float32).T           # [E, F]
        c = wt.reshape(2, 128, 4, 128)              # [kc, p, fc, j]
        out[:, wi] = c.transpose(1, 0, 2, 3)
    return out.reshape(128, -1)


def _expand_bn(v1, v2):
    """bn1/bn2 [F] -> [128, 24] per (slot, fc) expanded."""
    out = np.empty((128, 6, 4), np.float32)
    for slot in range(6):
        src = v1 if slot < 2 else v2
        out[:, slot, :] = np.asarray(src, np.float32).reshape(4, 128).T
    return out.reshape(128, 24)


_CACHE = {}


def _get_nc(bl):
    if bl not in _CACHE:
        _CACHE[bl] = build_nc(bl)
    return _CACHE[bl]


def _compact(tbl_bf16, slots_idx, nrows):
    """Compacted per-core table + remapped int16 indices.

    slots_idx: [bl, nslots] int into the full table. Returns
    (table [nrows, E] bf16, idx [bl, nslots] int16).
    """
    uniq, inv = np.unique(slots_idx, return_inverse=True)
    assert uniq.size <= nrows
    tbl_c = np.zeros((nrows, E), dtype=tbl_bf16.dtype)
    tbl_c[:uniq.size] = tbl_bf16[uniq]
    return tbl_c, inv.reshape(slots_idx.shape).astype(np.int16)


def make_in_maps(x_batch, emb_roles, emb_values, conv1_w, conv2_w,
                 bn1_gamma, bn1_beta, bn2_gamma, bn2_beta, fc_w, fc_b,
                 bl, nb=1024):
    nchunk = bl // nb
    er = np.asarray(emb_roles, np.float32).astype(ml_dtypes.bfloat16)
    ev = np.asarray(emb_values, np.float32).astype(ml_dtypes.bfloat16)
    shared = {
        "w_all": _chunk_weights(conv1_w, conv2_w),
        "fcw": np.asarray(fc_w, np.float32).reshape(4, 128).T
                 .astype(ml_dtypes.bfloat16),
        "fcb": np.asarray(fc_b, np.float32).reshape(1, 1),
        "gamma_x": _expand_bn(bn1_gamma, bn2_gamma),
        "beta_x": _expand_bn(bn1_beta, bn2_beta),
    }
    xb = np.asarray(x_batch).astype(np.int64)
    in_maps = []
    for c in range(CORES):
        xs = xb[c * bl:(c + 1) * bl]
        roles = xs[:, 0::2]
        values = xs[:, 1::2]
        r_slots = roles[:, [0, 2, 3, 4, 5]]
        v_slots = values[:, [0, 1, 2, 3, 4, 5]]
        r_tbl, r_idx = _compact(er, r_slots, RSLOTS * bl)
        v_tbl, v_idx = _compact(ev, v_slots, VSLOTS * bl)
        m = dict(shared)
        m.update({
            "r_tbl": r_tbl, "v_tbl": v_tbl,
            "idx_all": _prep_idx_packed(r_idx, v_idx, nb, nchunk),
        })
        in_maps.append(m)
    return in_maps


def kernel(x_batch, arity, emb_roles, emb_values,
           conv1_w, conv1_b, bn1_gamma, bn1_beta,
           conv2_w, conv2_b, bn2_gamma, bn2_beta, fc_w, fc_b):
    # conv biases cancel exactly under training-mode batchnorm.
    bl = np.asarray(x_batch).shape[0] // CORES
    nc = _get_nc(bl)
    in_maps = make_in_maps(x_batch, emb_roles, emb_values, conv1_w, conv2_w,
                           bn1_gamma, bn1_beta, bn2_gamma, bn2_beta,
                           fc_w, fc_b, bl)
    res = run_bass_kernel_spmd(nc, in_maps, core_ids=list(range(CORES)))
    out = np.concatenate([res.results[c]["out"].reshape(bl, 1)
                          for c in range(CORES)], 0)
    return out.astype(np.float32)


# revision 12
# speedup vs baseline: 1.1092x; 1.0399x over previous
"""Trainium2 Bass kernel for nn_HINGE_56985626083396 (dense_cnn) — v2.

Data-parallel over batch on 8 NeuronCores. Differences vs v1:
  * per-core vocab compaction on host: each core references at most
    5*bl role rows / 6*bl value rows (< 32768), so indices fit int16
    against a per-core compacted table -> ONE dma_gather per table per
    chunk (v1 needed lo/hi double gathers + DVE merge adds).
    (num_swdge_queues=2 silently corrupts gathers on HW -- single queue.)
  * batch sums for BN means come free from the PSUM->SBUF combines
    (ACT copy accum / DVE scalar_tensor_tensor accum), replacing v1's
    activation-engine copy-accum over inputs + S1 matmul machinery.
  * nb=1024 chunks; window outputs packed per (chunk,fc) into one
    [128, 6*nb] tile -> ONE spill DMA per (chunk,fc), one reload per
    (block,fc) in phase 2.

Per core:
  phase 1 (per 1024-row chunk): gather embeddings (bf16, transposed
    layout); per fc quarter: conv1 windows o0/o1 formed whole in PSUM
    (shared W1r*fr0 term recomputed), conv2 shared B2 + per-k partial
    sums on PE; ACT/DVE combine to bf16 + per-(slot,fc) batch sums;
    squares with accum (split ACT/DVE) -> sum-of-squares; one spill DMA.
  AllReduce (8 cores) of the [128,48] BN sufficient statistics.
  phase 2 (per 1024-row block): reload, BN affine on ACT, min over 6
    windows + relu on DVE, FC dot on PE -> [bl] f32.
"""

import numpy as np
import ml_dtypes

from concourse import bass, bacc, mybir
import concourse.tile as tile
from concourse.bass_utils import run_bass_kernel_spmd
from concourse.library_config import mlp

CORES = 8
B_FULL = 32768
E = 256
F = 512
ARITY = 6
BN_EPS = 1e-5
RSLOTS = 5   # fr0, kr0..kr3     (roles table)
VSLOTS = 6   # fv0, fv1, kv0..3  (values table)

bf16 = mybir.dt.bfloat16
f32 = mybir.dt.float32
i16 = mybir.dt.int16
AF = mybir.ActivationFunctionType
OP = mybir.AluOpType

# weight ids: 0=W1v 1=W1r 2=W2_fv0 3=W2_fr0 4=W2_fv1 5=W2_kr 6=W2_kv
# role slot ids: fr0=0 kr_k=1+k ; value slot ids: fv0=0 fv1=1 kv_k=2+k


def build_nc(bl, nb=1024, use_cc=True, phase2=True):
    """Build the per-core Bass module. bl = per-core batch."""
    nchunk = bl // nb
    nrc = RSLOTS * bl   # compacted roles table rows (upper bound)
    nvc = VSLOTS * bl   # compacted values table rows
    nc = bacc.Bacc("TRN2", target_bir_lowering=False, debug=False,
                   num_devices=CORES)

    r_tbl = nc.dram_tensor("r_tbl", [nrc, E], bf16, kind="ExternalInput")
    v_tbl = nc.dram_tensor("v_tbl", [nvc, E], bf16, kind="ExternalInput")
    # host pre-chunked: [p, (w, kc, fc, j)]
    w_all_d = nc.dram_tensor("w_all", [128, 7 * 2 * 4 * 128], bf16,
                             kind="ExternalInput")
    fcw_d = nc.dram_tensor("fcw", [128, 4], bf16, kind="ExternalInput")
    fcb_d = nc.dram_tensor("fcb", [1, 1], f32, kind="ExternalInput")
    gam_d = nc.dram_tensor("gamma_x", [128, 24], f32, kind="ExternalInput")
    bet_d = nc.dram_tensor("beta_x", [128, 24], f32, kind="ExternalInput")
    NRI = RSLOTS * nb
    NVI = VSLOTS * nb
    WR, WV = NRI // 16, NVI // 16
    idx_d = nc.dram_tensor("idx_all", [128, nchunk * (WR + WV)], i16,
                           kind="ExternalInput")
    out_d = nc.dram_tensor("out", [1, bl], f32, kind="ExternalOutput")
    # [fc][p][(chunk, slot6, nb)] -- one contiguous 12KB/partition write
    # per (chunk, fc)
    spill = nc.dram_tensor("spill", [4, 128, nchunk * 6 * nb], bf16)

    nc.gpsimd.load_library(mlp)

    with tile.TileContext(nc) as tc:
        with tc.tile_pool(name="wp", bufs=1) as wp, \
             tc.tile_pool(name="statp", bufs=1) as statp:
            w_sb = wp.tile([128, 7 * 2 * 4 * 128], bf16)
            nc.sync.dma_start(out=w_sb[:], in_=w_all_d[:])
            fcw_sb = wp.tile([128, 4], bf16)
            nc.sync.dma_start(out=fcw_sb[:], in_=fcw_d[:])
            fcb_sb = wp.tile([1, 1], f32)
            nc.sync.dma_start(out=fcb_sb[:], in_=fcb_d[:])
            gam_sb = wp.tile([128, 24], f32)
            nc.sync.dma_start(out=gam_sb[:], in_=gam_d[:])
            bet_sb = wp.tile([128, 24], f32)
            nc.sync.dma_start(out=bet_sb[:], in_=bet_d[:])

            def w(wi, kc, fc):
                off = ((wi * 2 + kc) * 4 + fc) * 128
                return w_sb[:, off:off + 128]

            # per-(slot6,fc,chunk,half) batch sums; per-(slot6,fc,chunk)
            # sums of squares
            sumc = statp.tile([128, 6 * 4 * nchunk * 2], f32)
            s2c = statp.tile([128, 6 * 4 * nchunk], f32)

            # ---------------- phase 1 ----------------
            with tc.tile_pool(name="idxp", bufs=1) as idxp, \
                 tc.tile_pool(name="xp", bufs=2) as xp, \
                 tc.tile_pool(name="scrp", bufs=2) as scrp, \
                 tc.tile_pool(name="outp", bufs=3) as outp, \
                 tc.tile_pool(name="po", bufs=2, space="PSUM") as po, \
                 tc.tile_pool(name="psg", bufs=2, space="PSUM") as psg:
                idx_sb = idxp.tile([128, nchunk * (WR + WV)], i16)
                nc.sync.dma_start(out=idx_sb[:], in_=idx_d[:])

                for ch in range(nchunk):
                    co = ch * (WR + WV)
                    rl = idx_sb[:, co:co + WR]
                    vl = idx_sb[:, co + WR:co + WR + WV]

                    xr = xp.tile([128, 2, NRI], bf16, tag="xr")
                    nc.gpsimd.dma_gather(xr[:], r_tbl[:], rl, NRI, NRI, E,
                                         transpose=True, single_packet=False)
                    xv = xp.tile([128, 2, NVI], bf16, tag="xv")
                    nc.gpsimd.dma_gather(xv[:], v_tbl[:], vl, NVI, NVI, E,
                                         transpose=True, single_packet=False)

                    def xs_r(s, kc):
                        return xr[:, kc, s * nb:(s + 1) * nb]

                    def xs_v(s, kc):
                        return xv[:, kc, s * nb:(s + 1) * nb]

                    HB = nb // 2  # PSUM ops stay within one 512-f32 bank

                    def mm_group2(t0, t1, units):
                        """units: list of (wi, x-slice-fn, slot). Accumulate
                        sum_u W[u]*x[u] into t0/t1 (batch halves). Both
                        halves run back-to-back under one weight load."""
                        n = len(units) * 2
                        i = 0
                        for (wi, xs, s) in units:
                            for kc in range(2):
                                for h, tt in ((0, t0), (1, t1)):
                                    nc.tensor.matmul(
                                        out=tt[:],
                                        lhsT=w(wi, kc, fc),
                                        rhs=xs(s, kc)[:, h * HB:(h + 1) * HB],
                                        start=(i == 0), stop=(i == n - 1))
                                i += 1

                    def emit_squares(obuf_prev, fc_prev):
                        # sums of squares, lagged one fc so the ACT queue
                        # never delays the PSUM-freeing copies (ACT is a
                        # strict 8-deep FIFO).  (tensor_tensor_reduce and
                        # Identity-with-AP-affine both wedge real HW.)
                        for s6 in range(6):
                            col = (s6 * 4 + fc_prev) * nchunk + ch
                            sqs = scrp.tile([128, nb], bf16, tag="sqs")
                            nc.scalar.activation(
                                out=sqs[:], in_=obuf_prev[:, s6, :],
                                func=AF.Square,
                                accum_out=s2c[:, col:col + 1])

                    prev = None
                    for fc in range(4):
                        obuf = outp.tile([128, 6, nb], bf16, tag="obuf")

                        def ob(s6, h):
                            return obuf[:, s6, h * HB:(h + 1) * HB]

                        def scol(s6, h):
                            c = (((s6 * 4 + fc) * nchunk) + ch) * 2 + h
                            return sumc[:, c:c + 1]

                        # conv1 windows formed whole in PSUM:
                        # o0 = W1v*fv0 + W1r*fr0 ; o1 = W1v*fv1 + W1r*fr0
                        # (shared W1r*fr0 recomputed -- cheaper than a third
                        #  PSUM tile + DVE combine under the bank budget)
                        for s6 in range(2):
                            t0 = po.tile([128, HB], f32, tag="po0")
                            t1 = po.tile([128, HB], f32, tag="po1")
                            mm_group2(t0, t1, [(0, xs_v, s6), (1, xs_r, 0)])
                            for h, tt in ((0, t0), (1, t1)):
                                nc.vector.tensor_scalar(
                                    out=ob(s6, h), in0=tt[:],
                                    scalar1=1.0, scalar2=0.0,
                                    op0=OP.mult, op1=OP.add,
                                    accum_out=scol(s6, h))

                        # conv2 shared part: B2 = W2fv0*fv0 + W2fr0*fr0 + W2fv1*fv1
                        b2sb = scrp.tile([128, nb], bf16, tag="b2sb")
                        t0 = po.tile([128, HB], f32, tag="po0")
                        t1 = po.tile([128, HB], f32, tag="po1")
                        mm_group2(t0, t1, [(2, xs_v, 0), (3, xs_r, 0),
                                           (4, xs_v, 1)])
                        for h, tt in ((0, t0), (1, t1)):
                            nc.scalar.copy(out=b2sb[:, h * HB:(h + 1) * HB],
                                           in_=tt[:])
                        if prev is not None:
                            emit_squares(*prev)

                        # conv2 per-k: g_k = W2kr*kr_k + W2kv*kv_k (+B2)
                        for k in range(4):
                            g0 = psg.tile([128, HB], f32, tag="g0")
                            g1 = psg.tile([128, HB], f32, tag="g1")
                            mm_group2(g0, g1, [(5, xs_r, 1 + k),
                                               (6, xs_v, 2 + k)])
                            for h, gg in ((0, g0), (1, g1)):
                                nc.vector.scalar_tensor_tensor(
                                    out=ob(2 + k, h), in0=gg[:], scalar=1.0,
                                    in1=b2sb[:, h * HB:(h + 1) * HB],
                                    op0=OP.mult, op1=OP.add,
                                    accum_out=scol(2 + k, h))

                        eng = nc.sync if fc % 2 == 0 else nc.scalar
                        eng.dma_start(
                            out=spill[fc, :, (ch * 6) * nb:(ch * 6 + 6) * nb],
                            in_=obuf[:])
                        prev = (obuf, fc)
                    emit_squares(*prev)

            # ---------------- stats + allreduce ----------------
            with tc.tile_pool(name="fsp", bufs=1) as fsp, \
                 tc.tile_pool(name="dramp", bufs=1, space="DRAM") as dramp:
                stats = fsp.tile([128, 48], f32)
                nc.vector.tensor_reduce(
                    out=stats[:, 0:24],
                    in_=sumc[:].rearrange("p (a c) -> p a c", c=nchunk * 2),
                    axis=mybir.AxisListType.X, op=OP.add)
                nc.vector.tensor_reduce(
                    out=stats[:, 24:48],
                    in_=s2c[:].rearrange("p (a c) -> p a c", c=nchunk),
                    axis=mybir.AxisListType.X, op=OP.add)

                cc_in = dramp.tile([128, 48], f32)
                cc_out = dramp.tile([128, 48], f32)
                nc.sync.dma_start(out=cc_in[:], in_=stats[:])
                ar = fsp.tile([128, 48], f32)
                if use_cc:
                    nc.gpsimd.collective_compute(
                        "AllReduce", OP.add,
                        replica_groups=[list(range(CORES))],
                        ins=[cc_in[:].opt()], outs=[cc_out[:].opt()])
                    nc.sync.dma_start(out=ar[:], in_=cc_out[:])
                else:
                    nc.sync.dma_start(out=ar[:], in_=cc_in[:])

                inv_n = 1.0 / (bl * CORES) if use_cc else 1.0 / bl
                mean = fsp.tile([128, 24], f32)
                nc.scalar.mul(out=mean[:], in_=ar[:, 0:24], mul=inv_n)
                ex2 = fsp.tile([128, 24], f32)
                nc.scalar.mul(out=ex2[:], in_=ar[:, 24:48], mul=inv_n)
                var = fsp.tile([128, 24], f32)
                nc.vector.tensor_tensor(out=var[:], in0=mean[:], in1=mean[:],
                                        op=OP.mult)
                nc.vector.tensor_tensor(out=var[:], in0=ex2[:], in1=var[:],
                                        op=OP.subtract)
                nc.vector.tensor_scalar_add(out=var[:], in0=var[:],
                                            scalar1=BN_EPS)
                std = fsp.tile([128, 24], f32)
                nc.scalar.activation(out=std[:], in_=var[:], func=AF.Sqrt,
                                     bias=0.0, scale=1.0)
                rstd = fsp.tile([128, 24], f32)
                nc.vector.reciprocal(out=rstd[:], in_=std[:])
                avec = fsp.tile([128, 24], f32)
                nc.vector.tensor_tensor(out=avec[:], in0=rstd[:], in1=gam_sb[:],
                                        op=OP.mult)
                cvec = fsp.tile([128, 24], f32)
                nc.vector.tensor_tensor(out=cvec[:], in0=mean[:], in1=avec[:],
                                        op=OP.mult)
                nc.vector.tensor_tensor(out=cvec[:], in0=bet_sb[:], in1=cvec[:],
                                        op=OP.subtract)

                # ---------------- phase 2 ----------------
                with tc.tile_pool(name="ldp", bufs=3) as ldp, \
                     tc.tile_pool(name="yp", bufs=4) as yp, \
                     tc.tile_pool(name="mp", bufs=2) as mp, \
                     tc.tile_pool(name="mrp", bufs=8) as mrp, \
                     tc.tile_pool(name="orow", bufs=1) as orow, \
                     tc.tile_pool(name="ps2", bufs=2, space="PSUM") as ps2:
                    outrow = orow.tile([1, bl], f32)
                    if not phase2:
                        dbg = fsp.tile([1, bl], f32)
                        nc.vector.tensor_copy(out=dbg[:, 0:24], in_=avec[:1, :])
                        nc.sync.dma_start(out=out_d[:], in_=dbg[:])
                    for ch in (range(nchunk) if phase2 else []):
                        bo = ch * nb
                        minres = []
                        for fc in range(4):
                            ld = ldp.tile([128, 6, nb], bf16, tag="ld")
                            eng = nc.sync if fc % 2 == 0 else nc.scalar
                            eng.dma_start(
                                out=ld[:],
                                in_=spill[fc, :, (ch * 6) * nb:(ch * 6 + 6) * nb])
                            acc = None
                            for s6 in range(6):
                                col = s6 * 4 + fc
                                y = yp.tile([128, nb], bf16, tag="y")
                                nc.vector.tensor_scalar(
                                    out=y[:], in0=ld[:, s6, :],
                                    scalar1=avec[:, col:col + 1],
                                    scalar2=cvec[:, col:col + 1],
                                    op0=OP.mult, op1=OP.add)
                                if acc is None:
                                    acc = y
                                elif s6 < 5:
                                    m2 = mp.tile([128, nb], bf16, tag="m")
                                    nc.vector.tensor_tensor(
                                        out=m2[:], in0=acc[:], in1=y[:],
                                        op=OP.min)
                                    acc = m2
                                else:
                                    mr = mrp.tile([128, nb], bf16, tag="mr")
                                    nc.vector.tensor_tensor(
                                        out=mr[:], in0=acc[:], in1=y[:],
                                        op=OP.min)
                                    minres.append(mr)
                        fcsub = 512
                        for sub in range(0, nb, fcsub):
                            pfc = ps2.tile([1, fcsub], f32, tag="pfc")
                            for fc in range(4):
                                relu = mrp.tile([128, fcsub], bf16, tag="relu")
                                nc.scalar.activation(
                                    out=relu[:],
                                    in_=minres[fc][:, sub:sub + fcsub],
                                    func=AF.Relu, bias=0.0, scale=1.0)
                                nc.tensor.matmul(out=pfc[:],
                                                 lhsT=fcw_sb[:, fc:fc + 1],
                                                 rhs=relu[:],
                                                 start=(fc == 0), stop=(fc == 3))
                            nc.vector.tensor_scalar_add(
                                out=outrow[:, bo + sub:bo + sub + fcsub],
                                in0=pfc[:], scalar1=fcb_sb[:1, :1])
                    if phase2:
                        nc.sync.dma_start(out=out_d[:], in_=outrow[:])

    nc.compile()
    return nc


# ---------------- host side ----------------

def _wrap16(a):
    """int16 index array -> [128, n/16] wrapped layout."""
    a = a.astype(np.int16)
    w = a.reshape(-1, 16).T
    return np.tile(w, (8, 1))


def _prep_idx_packed(r_idx, v_idx, nb, nchunk):
    """r_idx/v_idx: [bl, nslots] int (< 32768). -> [128, nchunk*(WR+WV)]
    where per chunk the roles block then values block are laid out
    column-contiguous (wrapped in 16 partitions, replicated to 128)."""
    cols = []
    for ch in range(nchunk):
        for slots_idx in (r_idx, v_idx):
            blk = slots_idx[ch * nb:(ch + 1) * nb]   # [nb, nslots]
            flat = blk.T.reshape(-1)                  # slot-major
            cols.append(_wrap16(flat))
    return np.concatenate(cols, 1)


def _chunk_weights(conv1_w, conv2_w):
    """-> [128, 7*2*4*128] bf16 host-chunked transposed weights."""
    ws = [conv1_w[:, 0, :], conv1_w[:, 1, :],
          conv2_w[:, 0, :], conv2_w[:, 1, :], conv2_w[:, 2, :],
          conv2_w[:, 3, :], conv2_w[:, 4, :]]
    out = np.empty((128, 7, 2, 4, 128), dtype=ml_dtypes.bfloat16)
    for wi, wm in enumerate(ws):
        wt = np.asarray(wm, np.float32).T           # [E, F]
        c = wt.reshape(2, 128, 4, 128)              # [kc, p, fc, j]
        out[:, wi] = c.transpose(1, 0, 2, 3)
    return out.reshape(128, -1)


def _expand_bn(v1, v2):
    """bn1/bn2 [F] -> [128, 24] per (slot, fc) expanded."""
    out = np.empty((128, 6, 4), np.float32)
    for slot in range(6):
        src = v1 if slot < 2 else v2
        out[:, slot, :] = np.asarray(src, np.float32).reshape(4, 128).T
    return out.reshape(128, 24)


_CACHE = {}


def _get_nc(bl):
    if bl not in _CACHE:
        _CACHE[bl] = build_nc(bl)
    return _CACHE[bl]


def _compact(tbl_bf16, slots_idx, nrows):
    """Compacted per-core table + remapped int16 indices.

    slots_idx: [bl, nslots] int into the full table. Returns
    (table [nrows, E] bf16, idx [bl, nslots] int16).
    """
    uniq, inv = np.unique(slots_idx, return_inverse=True)
    assert uniq.size <= nrows
    tbl_c = np.zeros((nrows, E), dtype=tbl_bf16.dtype)
    tbl_c[:uniq.size] = tbl_bf16[uniq]
    return tbl_c, inv.reshape(slots_idx.shape).astype(np.int16)


def make_in_maps(x_batch, emb_roles, emb_values, conv1_w, conv2_w,
                 bn1_gamma, bn1_beta, bn2_gamma, bn2_beta, fc_w, fc_b,
                 bl, nb=1024):
    nchunk = bl // nb
    er = np.asarray(emb_roles, np.float32).astype(ml_dtypes.bfloat16)
    ev = np.asarray(emb_values, np.float32).astype(ml_dtypes.bfloat16)
    shared = {
        "w_all": _chunk_weights(conv1_w, conv2_w),
        "fcw": np.asarray(fc_w, np.float32).reshape(4, 128).T
                 .astype(ml_dtypes.bfloat16),
        "fcb": np.asarray(fc_b, np.float32).reshape(1, 1),
        "gamma_x": _expand_bn(bn1_gamma, bn2_gamma),
        "beta_x": _expand_bn(bn1_beta, bn2_beta),
    }
    xb = np.asarray(x_batch).astype(np.int64)
    in_maps = []
    for c in range(CORES):
        xs = xb[c * bl:(c + 1) * bl]
        roles = xs[:, 0::2]
        values = xs[:, 1::2]
        r_slots = roles[:, [0, 2, 3, 4, 5]]
        v_slots = values[:, [0, 1, 2, 3, 4, 5]]
        r_tbl, r_idx = _compact(er, r_slots, RSLOTS * bl)
        v_tbl, v_idx = _compact(ev, v_slots, VSLOTS * bl)
        m = dict(shared)
        m.update({
            "r_tbl": r_tbl, "v_tbl": v_tbl,
            "idx_all": _prep_idx_packed(r_idx, v_idx, nb, nchunk),
        })
        in_maps.append(m)
    return in_maps


def kernel(x_batch, arity, emb_roles, emb_values,
           conv1_w, conv1_b, bn1_gamma, bn1_beta,
           conv2_w, conv2_b, bn2_gamma, bn2_beta, fc_w, fc_b):
    # conv biases cancel exactly under training-mode batchnorm.
    bl = np.asarray(x_batch).shape[0] // CORES
    nc = _get_nc(bl)
    in_maps = make_in_maps(x_batch, emb_roles, emb_values, conv1_w, conv2_w,
                           bn1_gamma, bn1_beta, bn2_gamma, bn2_beta,
                           fc_w, fc_b, bl)
    res = run_bass_kernel_spmd(nc, in_maps, core_ids=list(range(CORES)))
    out = np.concatenate([res.results[c]["out"].reshape(bl, 1)
                          for c in range(CORES)], 0)
    return out.astype(np.float32)
